# revision 13
# baseline (speedup 1.0000x reference)
"""Trainium2 Bass kernel for nn_MemoryEfficientAttnBlock (windowed attention block).

Reference computation (B=4, C=512, H=W=64, WS=32, NHEADS=8, GROUPS=32):
  h = GroupNorm(x) -> window partition (2x2 windows of 32x32) -> q,k,v 1x1 convs
  -> per-(window, head) softmax attention over n=1024 positions, d=64
  -> window reverse -> output 1x1 conv -> residual add.

Sharding: data-parallel across the 8 cores: core c handles batch c//2,
spatial half c%2 (rows hi*32..hi*32+31 = 2 windows of 32x32). Conv weights
replicated. GroupNorm statistics span the full batch, so each core also
reads the *other* half of its batch (stats only, no collectives).

Device-side design notes:
  - GroupNorm is folded into the QKV weights: h = A[c]*x + B[c] per
    (batch, channel) with A = rstd*gamma, B = beta - mu*A, so
    q = (wq.diag(A)) x + (wq B + bq) etc.  Stats via bn_stats/bn_aggr,
    16-channel group reduction via a tiny indicator matmul.
  - Scores are computed directly transposed, S^T[m,n] = k^T q, so the
    softmax needs no max-pass (|s*scale| < ~2 here) and no transposes:
    expS = exp(scale*S^T) feeds attn@V as the stationary-side operand.
  - v is produced pre-transposed (v^T[pos, c]) by swapping matmul operands.
    Its tile carries two ones-blocks; the attn@V stationary operand is
    [v_h | ones] (or [ones | v_h] for odd heads), so one matmul yields both
    the unnormalized output and the softmax row-sums on the other
    partition half.  A 1-input ACT copy shifts the row-sums across
    partitions, reciprocal_approx_fast inverts them, one DVE multiply
    normalizes.
  - Big matmuls run as float32r (full PE rate on fp32 data); q/k/expS/v
    use bf16 where it only touches the attention inner products.
"""

import numpy as np

import concourse.bass as bass
import concourse.tile as tile
from concourse import bacc, mybir
from concourse.bass_utils import run_bass_kernel_spmd

f32 = mybir.dt.float32
f32r = mybir.dt.float32r
bf16 = mybir.dt.bfloat16
FT = mybir.ActivationFunctionType
OP = mybir.AluOpType

B, C, H, W = 4, 512, 64, 64
WS, NHEADS, D = 32, 8, 64
GROUPS, EPS = 32, 1e-6
SCALE = 1.0 / 8.0          # 1/sqrt(D)
NCH = C // 128             # 4 channel chunks
NWIN = 2                   # windows per core
N = WS * WS                # 1024 positions per window
NPOS = NWIN * N            # 2048 positions per core
NCORES = 8


def _r(ap):
    return ap.bitcast(f32r)


def build_kernel(reps: int = 1, dbg: bool = False):
    """Build + compile the per-core Bass program. Returns the Bacc object."""
    nc = bacc.Bacc("TRN2", target_bir_lowering=False, debug=False,
                   num_devices=NCORES)

    xm_d = nc.dram_tensor("xm", [C, NPOS], f32r, kind="ExternalInput").ap()
    xo_d = nc.dram_tensor("xo", [C, NPOS], f32, kind="ExternalInput").ap()
    wq_d = nc.dram_tensor("wqT", [C, C], f32, kind="ExternalInput").ap()
    wk_d = nc.dram_tensor("wkT", [C, C], f32, kind="ExternalInput").ap()
    wv_d = nc.dram_tensor("wvT", [C, C], f32, kind="ExternalInput").ap()
    wo_d = nc.dram_tensor("woT", [C, C], f32r, kind="ExternalInput").ap()
    gsc_d = nc.dram_tensor("gscale", [128, NCH], f32, kind="ExternalInput").ap()
    gbi_d = nc.dram_tensor("gbias", [128, NCH], f32, kind="ExternalInput").ap()
    bq_d = nc.dram_tensor("bq", [128, NCH], f32, kind="ExternalInput").ap()
    bk_d = nc.dram_tensor("bk", [128, NCH], f32, kind="ExternalInput").ap()
    bo_d = nc.dram_tensor("bo", [128, NCH], f32, kind="ExternalInput").ap()
    bv_d = nc.dram_tensor("bv", [1, C], f32, kind="ExternalInput").ap()
    g_d = nc.dram_tensor("G", [128, 8], f32, kind="ExternalInput").ap()
    gt_d = nc.dram_tensor("Gt", [8, 128], f32, kind="ExternalInput").ap()
    out_d = nc.dram_tensor("out", [C, NPOS], f32, kind="ExternalOutput").ap()
    dbg_d = None
    if dbg:
        dbg_d = {
            "dAcol": nc.dram_tensor("dAcol", [128, NCH], f32, kind="ExternalOutput").ap(),
            "dBcol": nc.dram_tensor("dBcol", [128, NCH], f32, kind="ExternalOutput").ap(),
            "dbq2": nc.dram_tensor("dbq2", [128, NCH], f32, kind="ExternalOutput").ap(),
            "dbvb": nc.dram_tensor("dbvb", [128, C], f32, kind="ExternalOutput").ap(),
            "dmv0": nc.dram_tensor("dmv0", [128, 2], f32, kind="ExternalOutput").ap(),
            "dq0": nc.dram_tensor("dq0", [128, N], f32, kind="ExternalOutput").ap(),
            "dk0": nc.dram_tensor("dk0", [128, N], f32, kind="ExternalOutput").ap(),
            "dvt0": nc.dram_tensor("dvt0", [128, 1024], f32, kind="ExternalOutput").ap(),
            "des0": nc.dram_tensor("des0", [128, N], f32, kind="ExternalOutput").ap(),
            "dao0": nc.dram_tensor("dao0", [128, N], f32, kind="ExternalOutput").ap(),
            "dav": nc.dram_tensor("dav", [128, N], f32, kind="ExternalOutput").ap(),
            "drr": nc.dram_tensor("drr", [128, N], f32, kind="ExternalOutput").ap(),
            "dri": nc.dram_tensor("dri", [128, N], f32, kind="ExternalOutput").ap(),
        }

    with tile.TileContext(nc) as tc:
        with (
            tc.tile_pool(name="persist", bufs=1) as P,
            tc.tile_pool(name="xo_stream", bufs=2) as XO,
            tc.tile_pool(name="stats", bufs=2) as ST,
            tc.tile_pool(name="fold", bufs=1) as FP,
            tc.tile_pool(name="qk", bufs=1) as QK,
            tc.tile_pool(name="vt", bufs=1) as VT,
            tc.tile_pool(name="es", bufs=2) as ES,
            tc.tile_pool(name="ao", bufs=1) as AO,
            tc.tile_pool(name="rr", bufs=1) as RR,
            tc.tile_pool(name="ysb", bufs=2) as YP,
            tc.tile_pool(name="osb", bufs=2) as OS,
            tc.tile_pool(name="ps_proj", bufs=2, space="PSUM") as PSP,
            tc.tile_pool(name="ps_sc", bufs=2, space="PSUM") as PSS,
            tc.tile_pool(name="ps_av", bufs=1, space="PSUM") as PSA,
        ):
            # ---- persistent loads (once) ----
            x_sb = []
            for kc in range(NCH):
                t = P.tile([128, NPOS], f32r, tag=f"x{kc}")
                nc.sync.dma_start(out=t, in_=xm_d[kc * 128:(kc + 1) * 128, :])
                x_sb.append(t)
            worig = {}
            for nm, d in (("q", wq_d), ("k", wk_d), ("v", wv_d), ("o", wo_d)):
                worig[nm] = []
                for kc in range(NCH):
                    t = P.tile([128, C], f32r if nm == "o" else f32,
                               tag=f"w{nm}{kc}")
                    nc.sync.dma_start(out=t, in_=d[kc * 128:(kc + 1) * 128, :])
                    worig[nm].append(t)
            gsc = P.tile([128, NCH], f32, tag="gsc")
            nc.sync.dma_start(out=gsc, in_=gsc_d)
            gbi = P.tile([128, NCH], f32, tag="gbi")
            nc.sync.dma_start(out=gbi, in_=gbi_d)
            bqc = P.tile([128, NCH], f32, tag="bqc")
            nc.sync.dma_start(out=bqc, in_=bq_d)
            bkc = P.tile([128, NCH], f32, tag="bkc")
            nc.sync.dma_start(out=bkc, in_=bk_d)
            boc = P.tile([128, NCH], f32, tag="boc")
            nc.sync.dma_start(out=boc, in_=bo_d)
            bvr = P.tile([1, C], f32, tag="bvr")
            nc.sync.dma_start(out=bvr, in_=bv_d)
            Gm = P.tile([128, 8], f32, tag="Gm")
            nc.sync.dma_start(out=Gm, in_=g_d)
            Gt = P.tile([8, 128], f32, tag="Gt")
            nc.sync.dma_start(out=Gt, in_=gt_d)
            ones1 = P.tile([1, 128], f32, tag="ones1")
            nc.vector.memset(ones1, 1.0)

            for _ in range(reps):
                _body(nc, x_sb, worig, gsc, gbi, bqc, bkc, boc, bvr, Gm, Gt,
                      ones1, xo_d, out_d, XO, ST, FP, QK, VT, ES, AO, RR, YP,
                      OS, PSP, PSS, PSA, dbg_d)

    nc.compile()
    return nc


def _body(nc, x_sb, worig, gsc, gbi, bqc, bkc, boc, bvr, Gm, Gt, ones1,
          xo_d, out_d, XO, ST, FP, QK, VT, ES, AO, RR, YP, OS,
          PSP, PSS, PSA, dbg_d=None):

    def _dump(name, ap, cast_pool=None):
        if dbg_d is None or name not in dbg_d:
            return
        if ap.dtype == f32 or ap.dtype == f32r:
            nc.sync.dma_start(out=dbg_d[name],
                              in_=ap.bitcast(f32) if ap.dtype != f32 else ap)
            return
        fs = ap.free_size()
        for o in range(0, fs, 512):
            wdt = min(512, fs - o)
            st = cast_pool.tile([128, 512], f32, tag="y", name="dbgcast")
            nc.vector.tensor_copy(out=st[:ap.shape[0], :wdt], in_=ap[:, o:o + wdt])
            nc.sync.dma_start(out=dbg_d[name][:ap.shape[0], o:o + wdt],
                              in_=st[:ap.shape[0], :wdt])
    # ================= GroupNorm statistics =================
    # Per-channel mean/var over the full batch = own half + other half.
    mv = []   # [128, 2] per chunk: {mean, E[x^2]} per channel (e overwrites var)
    for kc in range(NCH):
        stats = ST.tile([128, 8, 6], f32, tag="bnstats")
        xr = x_sb[kc].bitcast(f32).rearrange("p (s f) -> p s f", f=512)
        for s in range(4):
            nc.vector.bn_stats(out=stats[:, s, :], in_=xr[:, s, :])
        for h in range(2):
            xo_t = XO.tile([128, N], f32, tag="xo")
            nc.sync.dma_start(
                out=xo_t, in_=xo_d[kc * 128:(kc + 1) * 128, h * N:(h + 1) * N])
            xor = xo_t.rearrange("p (s f) -> p s f", f=512)
            for s in range(2):
                nc.vector.bn_stats(out=stats[:, 4 + 2 * h + s, :],
                                   in_=xor[:, s, :])
        m = ST.tile([128, 2], f32, tag=f"mv{kc}")
        nc.vector.bn_aggr(out=m, in_=stats)
        # col1 := var + mean^2 = E[x^2]
        tmp = ST.tile([128, 1], f32, tag="musq")
        nc.vector.tensor_tensor(out=tmp, in0=m[:, 0:1], in1=m[:, 0:1],
                                op=OP.mult)
        nc.vector.tensor_tensor(out=m[:, 1:2], in0=m[:, 1:2], in1=tmp,
                                op=OP.add)
        if kc == 0:
            _dump("dmv0", m)
        mv.append(m)

    # group sums: [8 local groups, {mean,e} x 4 chunks]
    ps_g = PSP.tile([8, 8], f32, tag="pp")
    for kc in range(NCH):
        nc.tensor.matmul(ps_g[:, 2 * kc:2 * kc + 2], lhsT=Gm, rhs=mv[kc],
                         start=True, stop=True)
    gs = ST.tile([8, 8], f32, tag="gs")
    nc.scalar.copy(out=gs, in_=ps_g)
    mug = ST.tile([8, NCH], f32, tag="mug")
    rstd = ST.tile([8, NCH], f32, tag="rstd")
    vtmp = ST.tile([8, NCH], f32, tag="vtmp")
    for kc in range(NCH):
        nc.vector.tensor_scalar_mul(out=mug[:, kc:kc + 1],
                                    in0=gs[:, 2 * kc:2 * kc + 1],
                                    scalar1=1.0 / 16.0)
        nc.vector.tensor_scalar_mul(out=vtmp[:, kc:kc + 1],
                                    in0=gs[:, 2 * kc + 1:2 * kc + 2],
                                    scalar1=1.0 / 16.0)
    # var = E[x^2] - mu^2 ; rstd = exp(-0.5*ln(var + eps))
    musq = ST.tile([8, NCH], f32, tag="musq8")
    nc.vector.tensor_tensor(out=musq, in0=mug, in1=mug, op=OP.mult)
    nc.vector.tensor_tensor(out=vtmp, in0=vtmp, in1=musq, op=OP.subtract)
    eps8 = ST.tile([8, 1], f32, tag="eps8")
    nc.vector.memset(eps8, EPS)
    nc.scalar.activation(out=vtmp, in_=vtmp, func=FT.Ln, bias=eps8, scale=1.0)
    nc.scalar.activation(out=rstd, in_=vtmp, func=FT.Exp, scale=-0.5)

    # broadcast group stats back to channels; A/B per channel
    Acol = ST.tile([128, NCH], f32, tag="Acol")
    Bcol = ST.tile([128, NCH], f32, tag="Bcol")
    for kc in range(NCH):
        ps_mu = PSP.tile([128, 1], f32, tag="pp")
        nc.tensor.matmul(ps_mu, lhsT=Gt, rhs=mug[:, kc:kc + 1],
                         start=True, stop=True)
        ps_rs = PSP.tile([128, 1], f32, tag="pp")
        nc.tensor.matmul(ps_rs, lhsT=Gt, rhs=rstd[:, kc:kc + 1],
                         start=True, stop=True)
        nc.vector.tensor_tensor(out=Acol[:, kc:kc + 1], in0=ps_rs,
                                in1=gsc[:, kc:kc + 1], op=OP.mult)
        t = ST.tile([128, 1], f32, tag="btmp")
        nc.vector.tensor_tensor(out=t, in0=ps_mu, in1=Acol[:, kc:kc + 1],
                                op=OP.mult)
        nc.vector.tensor_tensor(out=Bcol[:, kc:kc + 1], in0=gbi[:, kc:kc + 1],
                                in1=t, op=OP.subtract)

    # v-path bias row: bias''[o] = sum_c B[c] wvT[c,o] + bv[o], broadcast
    ps_br = PSP.tile([1, C], f32, tag="pp")
    for kc in range(NCH):
        nc.tensor.matmul(ps_br, lhsT=Bcol[:, kc:kc + 1], rhs=worig["v"][kc],
                         start=(kc == 0), stop=(kc == NCH - 1))
    brow = ST.tile([1, C], f32, tag="brow")
    nc.scalar.copy(out=brow, in_=ps_br)
    nc.vector.tensor_tensor(out=brow, in0=brow, in1=bvr, op=OP.add)
    ps_bb = PSP.tile([128, C], f32, tag="pp")
    nc.tensor.matmul(ps_bb, lhsT=ones1, rhs=brow, start=True, stop=True)
    bvb = ST.tile([128, C], f32, tag="bvb")
    nc.vector.tensor_copy(out=bvb, in_=ps_bb)

    # q/k bias columns: bias'[o] = sum_c B[c] wT[c, o*] + b
    bq2 = ST.tile([128, NCH], f32, tag="bq2")
    bk2 = ST.tile([128, NCH], f32, tag="bk2")
    for nm, bcol, bout in (("q", bqc, bq2), ("k", bkc, bk2)):
        for oc in range(NCH):
            ps_b = PSP.tile([128, 1], f32, tag="pp")
            for kc in range(NCH):
                nc.tensor.matmul(ps_b,
                                 lhsT=worig[nm][kc][:, oc * 128:(oc + 1) * 128],
                                 rhs=Bcol[:, kc:kc + 1],
                                 start=(kc == 0), stop=(kc == NCH - 1))
            nc.vector.tensor_tensor(out=bout[:, oc:oc + 1], in0=ps_b,
                                    in1=bcol[:, oc:oc + 1], op=OP.add)

    _dump("dAcol", Acol)
    _dump("dBcol", Bcol)
    _dump("dbq2", bq2)
    _dump("dbvb", bvb)

    # fold A into wq/wk/wv (separate tiles; originals stay pristine)
    wfold = {}
    for nm in ("q", "k", "v"):
        wfold[nm] = []
        for kc in range(NCH):
            t = FP.tile([128, C], f32r, tag=f"f{nm}{kc}")
            nc.vector.tensor_scalar_mul(out=t, in0=worig[nm][kc],
                                        scalar1=Acol[:, kc:kc + 1])
            wfold[nm].append(t)

    # ================= main per-window pipeline =================
    for w in range(NWIN):
        base = w * N
        # --- projections ---
        q_sb = [QK.tile([128, N], bf16, tag=f"q{kc}", name=f"q{kc}") for kc in range(NCH)]
        k_sb = [QK.tile([128, N], bf16, tag=f"k{kc}", name=f"k{kc}") for kc in range(NCH)]
        for dst, wf, bcol in ((q_sb, wfold["q"], bq2), (k_sb, wfold["k"], bk2)):
            for oc in range(NCH):
                for pc in range(2):
                    ps = PSP.tile([128, 512], f32, tag="pp")
                    for kc in range(NCH):
                        nc.tensor.matmul(
                            ps,
                            lhsT=wf[kc][:, oc * 128:(oc + 1) * 128],
                            rhs=x_sb[kc][:, base + pc * 512:base + (pc + 1) * 512],
                            start=(kc == 0), stop=(kc == NCH - 1))
                    nc.vector.tensor_scalar(
                        out=dst[oc][:, pc * 512:(pc + 1) * 512], in0=ps,
                        scalar1=bcol[:, oc:oc + 1], scalar2=None, op0=OP.add)
        # vt tiles: per-head 128-col blocks; even head h: [v_h | ones],
        # odd head h: [ones | v_h] -> one matmul per head gives out^T on one
        # partition half and softmax row-sums on the other.
        vt = []
        for mc in range(8):
            t = VT.tile([128, 1024], bf16, tag=f"vt{mc}")
            ps = PSP.tile([128, 512], f32, tag="pp")
            for kc in range(NCH):
                nc.tensor.matmul(
                    ps,
                    lhsT=x_sb[kc][:, base + mc * 128:base + (mc + 1) * 128],
                    rhs=wfold["v"][kc],
                    start=(kc == 0), stop=(kc == NCH - 1))
            ap3 = lambda a, off, step: bass.AP(
                tensor=a.tensor, offset=a.offset + off,
                ap=[a.ap[0], [step, 4], [1, 64]])
            # even heads h=2j: v at col h*128, ones at h*128+64
            nc.vector.tensor_tensor(out=ap3(t, 0, 256), in0=ap3(ps, 0, 128),
                                    in1=ap3(bvb, 0, 128), op=OP.add)
            # odd heads h=2j+1: ones at h*128, v at h*128+64
            nc.vector.tensor_tensor(out=ap3(t, 192, 256), in0=ap3(ps, 64, 128),
                                    in1=ap3(bvb, 64, 128), op=OP.add)
            nc.vector.memset(ap3(t, 64, 256), 1.0)
            nc.vector.memset(ap3(t, 128, 256), 1.0)
            vt.append(t)

        if w == 0:
            _dump("dq0", q_sb[0], YP)
            _dump("dk0", k_sb[0], YP)
            _dump("dvt0", vt[0], YP)

        # --- attention heads ---
        ao_sb = [AO.tile([128, N], f32r, tag=f"ao{kc}", name=f"ao{kc}") for kc in range(NCH)]
        for h in range(NHEADS):
            ck, po = h // 2, (h % 2) * 64
            # scores^T -> exp
            es_t = []
            for mc in range(8):
                ps_s = PSS.tile([128, N], f32, tag="pscore")
                for nh in range(2):
                    nc.tensor.matmul(
                        ps_s[:, nh * 512:(nh + 1) * 512],
                        lhsT=k_sb[ck][po:po + 64, mc * 128:(mc + 1) * 128],
                        rhs=q_sb[ck][po:po + 64, nh * 512:(nh + 1) * 512],
                        start=True, stop=True)
                et = ES.tile([128, N], bf16, tag=f"es{mc}")
                nc.scalar.activation(out=et, in_=ps_s, func=FT.Exp,
                                     scale=SCALE)
                if w == 0 and h == 0 and mc == 0:
                    _dump("des0", et, YP)
                es_t.append(et)
            # attn @ [v | ones]
            ps_av = PSA.tile([128, N], f32, tag="pav")
            for mc in range(8):
                lhsT = vt[mc][:, h * 128:(h + 1) * 128]
                for nh in range(2):
                    nc.tensor.matmul(ps_av[:, nh * 512:(nh + 1) * 512],
                                     lhsT=lhsT,
                                     rhs=es_t[mc][:, nh * 512:(nh + 1) * 512],
                                     start=(mc == 0), stop=(mc == 7))
            # normalize: out = out_un * (1/rowsum)
            if dbg_d is not None and w == 0 and h == 0:
                for o in range(0, N, 512):
                    st = YP.tile([128, 512], f32, tag="y", name="dbgav")
                    nc.vector.tensor_copy(out=st, in_=ps_av[:, o:o + 512])
                    nc.sync.dma_start(out=dbg_d["dav"][:, o:o + 512], in_=st)
            rr_t = RR.tile([128, N], f32, tag="rraw")
            ri_t = RR.tile([128, N], f32, tag="rinv")
            if h % 2 == 0:
                nc.scalar.copy(out=rr_t[0:64, :], in_=ps_av[64:128, :])
                nc.vector.reciprocal_approx_fast(out=ri_t[0:64, :],
                                                 in_=rr_t[0:64, :])
                nc.vector.tensor_tensor(out=ao_sb[ck][po:po + 64, :],
                                        in0=ps_av[0:64, :], in1=ri_t[0:64, :],
                                        op=OP.mult)
                if dbg_d is not None and w == 0 and h == 0:
                    _dump("drr", rr_t)
                    _dump("dri", ri_t)
            else:
                # reciprocal_approx_fast requires base partition 0; compute
                # there and shift the result up with a 1-input copy.
                nc.scalar.copy(out=rr_t[0:64, :], in_=ps_av[0:64, :])
                nc.vector.reciprocal_approx_fast(out=ri_t[0:64, :],
                                                 in_=rr_t[0:64, :])
                nc.vector.tensor_copy(out=ri_t[64:128, :], in_=ri_t[0:64, :])
                nc.vector.tensor_tensor(out=ao_sb[ck][po:po + 64, :],
                                        in0=ps_av[64:128, :],
                                        in1=ri_t[64:128, :], op=OP.mult)

        if w == 0:
            _dump("dao0", ao_sb[0])

        # --- output projection + residual ---
        for oc in range(NCH):
            for nh in range(2):
                ps_y = PSP.tile([128, 512], f32, tag="pp")
                for kc in range(NCH):
                    nc.tensor.matmul(
                        ps_y,
                        lhsT=worig["o"][kc][:, oc * 128:(oc + 1) * 128],
                        rhs=ao_sb[kc][:, nh * 512:(nh + 1) * 512],
                        start=(kc == 0), stop=(kc == NCH - 1))
                y_t = YP.tile([128, 512], f32, tag="y")
                nc.scalar.activation(out=y_t, in_=ps_y, func=FT.Identity,
                                     bias=boc[:, oc:oc + 1])
                o_t = OS.tile([128, 512], f32, tag="osb")
                nc.vector.tensor_tensor(
                    out=o_t, in0=y_t,
                    in1=x_sb[oc].bitcast(f32)[:, base + nh * 512:base + (nh + 1) * 512],
                    op=OP.add)
                nc.sync.dma_start(
                    out=out_d[oc * 128:(oc + 1) * 128,
                              base + nh * 512:base + (nh + 1) * 512],
                    in_=o_t)


# ---------------- host-side marshalling ----------------

def _rasterize(xb_half):
    """[C, 32, 64] -> [C, 2048] in (window, row, col) raster order."""
    return np.ascontiguousarray(
        xb_half.reshape(C, WS, 2, WS).transpose(0, 2, 1, 3).reshape(C, NPOS))


def _unrasterize(y):
    """[C, 2048] -> [C, 32, 64]."""
    return y.reshape(C, 2, WS, WS).transpose(0, 2, 1, 3).reshape(C, WS, W)


_NC_CACHE = {}


def _get_nc(reps=1):
    if reps not in _NC_CACHE:
        _NC_CACHE[reps] = build_kernel(reps)
    return _NC_CACHE[reps]


def make_in_maps(x, norm_scale, norm_bias, wq, bq, wk, bk, wv, bv, wo, bo):
    x = np.asarray(x, dtype=np.float32)
    cols = lambda v: np.ascontiguousarray(
        np.asarray(v, np.float32).reshape(NCH, 128).T)
    G = np.zeros((128, 8), np.float32)
    for p in range(128):
        G[p, p // 16] = 1.0
    shared = {
        "wqT": np.ascontiguousarray(np.asarray(wq, np.float32).T),
        "wkT": np.ascontiguousarray(np.asarray(wk, np.float32).T),
        "wvT": np.ascontiguousarray(np.asarray(wv, np.float32).T),
        "woT": np.ascontiguousarray(np.asarray(wo, np.float32).T),
        "gscale": cols(norm_scale), "gbias": cols(norm_bias),
        "bq": cols(bq), "bk": cols(bk), "bo": cols(bo),
        "bv": np.asarray(bv, np.float32).reshape(1, C).copy(),
        "G": G, "Gt": np.ascontiguousarray(G.T),
    }
    in_maps = []
    for c in range(NCORES):
        b, hi = c // 2, c % 2
        xm = _rasterize(x[b, :, hi * WS:(hi + 1) * WS, :])
        xo = _rasterize(x[b, :, (1 - hi) * WS:(1 - hi + 1) * WS, :])
        in_maps.append({"xm": xm, "xo": xo, **shared})
    return in_maps


def kernel(**inputs):
    nc = _get_nc(1)
    in_maps = make_in_maps(**inputs)
    res = run_bass_kernel_spmd(nc, in_maps, list(range(NCORES)))
    out = np.empty((B, C, H, W), np.float32)
    for c in range(NCORES):
        b, hi = c // 2, c % 2
        out[b, :, hi * WS:(hi + 1) * WS, :] = _unrasterize(res.results[c]["out"])
    return out


# revision 27
# speedup vs baseline: 4.8940x; 4.8940x over previous
"""Trainium2 Bass kernel for nn_MemoryEfficientAttnBlock (windowed attention block).

Reference computation (B=4, C=512, H=W=64, WS=32, NHEADS=8, GROUPS=32):
  h = GroupNorm(x) -> window partition (2x2 windows of 32x32) -> q,k,v 1x1 convs
  -> per-(window, head) softmax attention over n=1024 positions, d=64
  -> window reverse -> output 1x1 conv -> residual add.

Sharding: data-parallel across the 8 cores: core c handles batch c//2,
spatial half c%2 (rows hi*32..hi*32+31 = 2 windows of 32x32). Conv weights
replicated. GroupNorm statistics span the full batch, so each core also
reads the *other* half of its batch (stats only, no collectives).

Device-side design notes:
  - GroupNorm is folded into the QKV weights: h = A[c]*x + B[c] per
    (batch, channel) with A = rstd*gamma, B = beta - mu*A, so
    q = (wq.diag(A)) x + (wq B + bq) etc.  Stats via bn_stats/bn_aggr,
    16-channel group reduction via a tiny indicator matmul.
  - Scores are computed directly transposed, S^T[m,n] = k^T q, so the
    softmax needs no max-pass (|s*scale| < ~2 here) and no transposes:
    expS = exp(scale*S^T) feeds attn@V as the stationary-side operand.
  - v is produced pre-transposed (v^T[pos, c]) by swapping matmul operands.
    Its tile carries two ones-blocks; the attn@V stationary operand is
    [v_h | ones] (or [ones | v_h] for odd heads), so one matmul yields both
    the unnormalized output and the softmax row-sums on the other
    partition half.  A 1-input ACT copy shifts the row-sums across
    partitions, reciprocal_approx_fast inverts them, one DVE multiply
    normalizes.
  - Big matmuls run as float32r (full PE rate on fp32 data); q/k/expS/v
    use bf16 where it only touches the attention inner products.
"""

import numpy as np

import concourse.bass as bass
import concourse.tile as tile
from concourse import bacc, mybir
from concourse.bass_utils import run_bass_kernel_spmd

f32 = mybir.dt.float32
f32r = mybir.dt.float32r
bf16 = mybir.dt.bfloat16
FT = mybir.ActivationFunctionType
OP = mybir.AluOpType

B, C, H, W = 4, 512, 64, 64
WS, NHEADS, D = 32, 8, 64
GROUPS, EPS = 32, 1e-6
SCALE = 1.0 / 8.0          # 1/sqrt(D)
NCH = C // 128             # 4 channel chunks
NWIN = 2                   # windows per core
N = WS * WS                # 1024 positions per window
NPOS = NWIN * N            # 2048 positions per core
NCORES = 8


def _r(ap):
    return ap.bitcast(f32r)


def build_kernel(reps: int = 1, dbg: bool = False, stage: int = 5,
                 loop_iters: int | None = None):
    """Build + compile the per-core Bass program. Returns the Bacc object."""
    nc = bacc.Bacc("TRN2", target_bir_lowering=False, debug=False,
                   num_devices=NCORES)

    xm_d = nc.dram_tensor("xm", [C, NPOS], f32r, kind="ExternalInput").ap()
    xo_d = nc.dram_tensor("xo", [C, NPOS], f32, kind="ExternalInput").ap()
    wq_d = nc.dram_tensor("wqT", [C, C], f32, kind="ExternalInput").ap()
    wk_d = nc.dram_tensor("wkT", [C, C], f32, kind="ExternalInput").ap()
    wv_d = nc.dram_tensor("wvT", [C, C], f32, kind="ExternalInput").ap()
    wo_d = nc.dram_tensor("woT", [C, C], f32, kind="ExternalInput").ap()
    gsc_d = nc.dram_tensor("gscale", [128, NCH], f32, kind="ExternalInput").ap()
    gbi_d = nc.dram_tensor("gbias", [128, NCH], f32, kind="ExternalInput").ap()
    bq_d = nc.dram_tensor("bq", [128, NCH], f32, kind="ExternalInput").ap()
    bk_d = nc.dram_tensor("bk", [128, NCH], f32, kind="ExternalInput").ap()
    bo_d = nc.dram_tensor("bo", [128, NCH], f32, kind="ExternalInput").ap()
    bv_d = nc.dram_tensor("bv", [1, C], f32, kind="ExternalInput").ap()
    g_d = nc.dram_tensor("G", [128, 8], f32, kind="ExternalInput").ap()
    gt_d = nc.dram_tensor("Gt", [8, 128], f32, kind="ExternalInput").ap()
    out_d = nc.dram_tensor("out", [C, NPOS], f32, kind="ExternalOutput").ap()
    dbg_d = None
    if dbg:
        dbg_d = {
            "dAcol": nc.dram_tensor("dAcol", [128, NCH], f32, kind="ExternalOutput").ap(),
            "dBcol": nc.dram_tensor("dBcol", [128, NCH], f32, kind="ExternalOutput").ap(),
            "dbq2": nc.dram_tensor("dbq2", [128, NCH], f32, kind="ExternalOutput").ap(),
            "dbvb": nc.dram_tensor("dbvb", [128, C], f32, kind="ExternalOutput").ap(),
            "dmv0": nc.dram_tensor("dmv0", [128, 2], f32, kind="ExternalOutput").ap(),
            "dq0": nc.dram_tensor("dq0", [128, N], f32, kind="ExternalOutput").ap(),
            "dk0": nc.dram_tensor("dk0", [128, N], f32, kind="ExternalOutput").ap(),
            "dvt0": nc.dram_tensor("dvt0", [128, 1024], f32, kind="ExternalOutput").ap(),
            "des0": nc.dram_tensor("des0", [128, N], f32, kind="ExternalOutput").ap(),
            "dao0": nc.dram_tensor("dao0", [128, N], f32, kind="ExternalOutput").ap(),
            "dav": nc.dram_tensor("dav", [128, N], f32, kind="ExternalOutput").ap(),
            "drr": nc.dram_tensor("drr", [128, N], f32, kind="ExternalOutput").ap(),
            "dri": nc.dram_tensor("dri", [128, N], f32, kind="ExternalOutput").ap(),
        }

    with tile.TileContext(nc) as tc:
        with (
            tc.tile_pool(name="persist", bufs=1) as P,
            tc.tile_pool(name="xo_stream", bufs=2) as XO,
            tc.tile_pool(name="stats", bufs=2) as ST,
            tc.tile_pool(name="fold", bufs=1) as FP,
            tc.tile_pool(name="qk", bufs=2) as QK,
            tc.tile_pool(name="vt", bufs=1) as VT,
            tc.tile_pool(name="es", bufs=2) as ES,
            tc.tile_pool(name="ao", bufs=1) as AO,
            tc.tile_pool(name="rr", bufs=1) as RR,
            tc.tile_pool(name="ysb", bufs=2) as YP,
            tc.tile_pool(name="osb", bufs=2) as OS,
            tc.tile_pool(name="ps_proj", bufs=2, space="PSUM") as PSP,
            tc.tile_pool(name="ps_sc", bufs=2, space="PSUM") as PSS,
            tc.tile_pool(name="ps_av", bufs=1, space="PSUM") as PSA,
        ):
            # ---- persistent loads (once) ----
            x_sb = []
            for kc in range(NCH):
                t = P.tile([128, NPOS], f32r, tag=f"x{kc}")
                nc.sync.dma_start(out=t, in_=xm_d[kc * 128:(kc + 1) * 128, :])
                x_sb.append(t)
            worig = {}
            for nm, d in (("q", wq_d), ("k", wk_d), ("v", wv_d)):
                worig[nm] = []
                for kc in range(NCH):
                    t = P.tile([128, C], f32, tag=f"w{nm}{kc}")
                    nc.sync.dma_start(out=t, in_=d[kc * 128:(kc + 1) * 128, :])
                    worig[nm].append(t)
            worig["o"] = []
            for kc in range(NCH):
                stg = ES.tile([128, C], f32, tag=f"es{kc}", name=f"wostg{kc}")
                nc.sync.dma_start(out=stg, in_=wo_d[kc * 128:(kc + 1) * 128, :])
                t = P.tile([128, C], bf16, tag=f"wo{kc}")
                nc.vector.tensor_copy(out=t, in_=stg)
                worig["o"].append(t)
            gsc = P.tile([128, NCH], f32, tag="gsc")
            nc.sync.dma_start(out=gsc, in_=gsc_d)
            gbi = P.tile([128, NCH], f32, tag="gbi")
            nc.sync.dma_start(out=gbi, in_=gbi_d)
            bqc = P.tile([128, NCH], f32, tag="bqc")
            nc.sync.dma_start(out=bqc, in_=bq_d)
            bkc = P.tile([128, NCH], f32, tag="bkc")
            nc.sync.dma_start(out=bkc, in_=bk_d)
            boc = P.tile([128, NCH], f32, tag="boc")
            nc.sync.dma_start(out=boc, in_=bo_d)
            bvr = P.tile([1, C], f32, tag="bvr")
            nc.sync.dma_start(out=bvr, in_=bv_d)
            Gm = P.tile([128, 8], f32, tag="Gm")
            nc.sync.dma_start(out=Gm, in_=g_d)
            Gt = P.tile([8, 128], f32, tag="Gt")
            nc.sync.dma_start(out=Gt, in_=gt_d)
            ones1 = P.tile([1, 128], f32, tag="ones1")
            nc.vector.memset(ones1, 1.0)

            def _reps():
                for _ in range(reps):
                    _body(nc, x_sb, worig, gsc, gbi, bqc, bkc, boc, bvr, Gm,
                          Gt, ones1, xo_d, out_d, XO, ST, FP, QK, VT, ES, AO,
                          RR, YP, OS, PSP, PSS, PSA, dbg_d, stage)

            if loop_iters is None:
                _reps()
            else:
                with tc.For_i(0, loop_iters, 1):
                    _reps()

    nc.compile()
    return nc


def _body(nc, x_sb, worig, gsc, gbi, bqc, bkc, boc, bvr, Gm, Gt, ones1,
          xo_d, out_d, XO, ST, FP, QK, VT, ES, AO, RR, YP, OS,
          PSP, PSS, PSA, dbg_d=None, stage=5):

    def _dump(name, ap, cast_pool=None):
        if dbg_d is None or name not in dbg_d:
            return
        if ap.dtype == f32 or ap.dtype == f32r:
            nc.sync.dma_start(out=dbg_d[name],
                              in_=ap.bitcast(f32) if ap.dtype != f32 else ap)
            return
        fs = ap.free_size()
        for o in range(0, fs, 512):
            wdt = min(512, fs - o)
            st = cast_pool.tile([128, 512], f32, tag="y", name="dbgcast")
            nc.vector.tensor_copy(out=st[:ap.shape[0], :wdt], in_=ap[:, o:o + wdt])
            nc.sync.dma_start(out=dbg_d[name][:ap.shape[0], o:o + wdt],
                              in_=st[:ap.shape[0], :wdt])
    if stage <= 0:
        z = ST.tile([128, 4], f32, tag="z0")
        nc.vector.memset(z, 0.0)
        nc.sync.dma_start(out=out_d[0:128, 0:4], in_=z)
        return

    # ================= GroupNorm statistics =================
    # Per-channel mean/var over the full batch = own half + other half.
    # The other half streams in as bf16 (gpsimd cast-DMA) for stats only.
    mv = ST.tile([128, 2 * NCH], f32, tag="mv")  # cols 2k,2k+1 = {mean, E[x^2]}
    xo_tiles = []
    for kc in range(NCH):
        for h in range(2):
            xo_t = ES.tile([128, N], bf16, tag=f"es{2 * kc + h}",
                           name=f"xo{2 * kc + h}")
            nc.gpsimd.dma_start(
                out=xo_t, in_=xo_d[kc * 128:(kc + 1) * 128, h * N:(h + 1) * N])
            xo_tiles.append(xo_t)
    statst = []
    for kc in range(NCH):
        stats = ST.tile([128, 8, 6], f32, tag=f"bn{kc}", name=f"bn{kc}")
        xr = x_sb[kc].bitcast(f32).rearrange("p (s f) -> p s f", f=512)
        for s in range(4):
            nc.vector.bn_stats(out=stats[:, s, :], in_=xr[:, s, :])
        statst.append(stats)
    if stage == 11:
        z = ST.tile([128, 6], f32, tag="z1")
        nc.vector.tensor_copy(out=z, in_=statst[0][:, 0, :])
        nc.sync.dma_start(out=out_d[0:128, 0:6], in_=z)
        return
    for kc in range(NCH):
        stats = statst[kc]
        for h in range(2):
            xor = xo_tiles[2 * kc + h].rearrange("p (s f) -> p s f", f=512)
            for s in range(2):
                nc.vector.bn_stats(out=stats[:, 4 + 2 * h + s, :],
                                   in_=xor[:, s, :])
    if stage == 12:
        z = ST.tile([128, 6], f32, tag="z1")
        nc.vector.tensor_copy(out=z, in_=statst[0][:, 5, :])
        nc.sync.dma_start(out=out_d[0:128, 0:6], in_=z)
        return
    for kc in range(NCH):
        nc.vector.bn_aggr(out=mv[:, 2 * kc:2 * kc + 2], in_=statst[kc])
    # odd cols := var + mean^2 = E[x^2]
    mvr = mv.rearrange("p (k two) -> p k two", two=2)
    msq = ST.tile([128, NCH], f32, tag="msq")
    nc.vector.tensor_tensor(out=msq, in0=mvr[:, :, 0], in1=mvr[:, :, 0],
                            op=OP.mult)
    nc.vector.tensor_tensor(out=mvr[:, :, 1], in0=mvr[:, :, 1], in1=msq,
                            op=OP.add)
    if dbg_d is not None:
        _dump("dmv0", mv[:, 0:2])

    # group sums: one matmul -> [8 local groups, (mean,e) x 4 chunks]
    ps_g = PSS.tile([8, 2 * NCH], f32, tag="pscore", name="ps_g")
    nc.tensor.matmul(ps_g, lhsT=Gm, rhs=mv, start=True, stop=True)
    # mr: cols 0:4 = mu_g, cols 4:8 = rstd_g  (per chunk)
    mr = ST.tile([8, 2 * NCH], f32, tag="mr")
    psr = ps_g.rearrange("p (k two) -> p k two", two=2)
    nc.vector.tensor_scalar_mul(out=mr[:, 0:NCH], in0=psr[:, :, 0],
                                scalar1=1.0 / 16.0)
    nc.vector.tensor_scalar_mul(out=mr[:, NCH:2 * NCH], in0=psr[:, :, 1],
                                scalar1=1.0 / 16.0)
    # var = E[x^2] - mu^2 ; rstd = exp(-0.5*ln(var + eps))
    msq8 = ST.tile([8, NCH], f32, tag="msq8")
    nc.vector.tensor_tensor(out=msq8, in0=mr[:, 0:NCH], in1=mr[:, 0:NCH],
                            op=OP.mult)
    nc.vector.tensor_tensor(out=mr[:, NCH:2 * NCH], in0=mr[:, NCH:2 * NCH],
                            in1=msq8, op=OP.subtract)
    eps8 = ST.tile([8, 1], f32, tag="eps8")
    nc.vector.memset(eps8, EPS)
    nc.scalar.activation(out=mr[:, NCH:2 * NCH], in_=mr[:, NCH:2 * NCH],
                         func=FT.Ln, bias=eps8, scale=1.0)
    nc.scalar.activation(out=mr[:, NCH:2 * NCH], in_=mr[:, NCH:2 * NCH],
                         func=FT.Exp, scale=-0.5)

    # broadcast group stats back to channels (one matmul); A/B per channel
    ps_bc = PSS.tile([128, 2 * NCH], f32, tag="pscore", name="ps_bc")
    nc.tensor.matmul(ps_bc, lhsT=Gt, rhs=mr, start=True, stop=True)
    Acol = ST.tile([128, NCH], f32, tag="Acol")
    Bcol = ST.tile([128, NCH], f32, tag="Bcol")
    nc.vector.tensor_tensor(out=Acol, in0=ps_bc[:, NCH:2 * NCH], in1=gsc,
                            op=OP.mult)
    tb = ST.tile([128, NCH], f32, tag="tb")
    nc.vector.tensor_tensor(out=tb, in0=ps_bc[:, 0:NCH], in1=Acol, op=OP.mult)
    nc.vector.tensor_tensor(out=Bcol, in0=gbi, in1=tb, op=OP.subtract)

    if stage == 13:
        nc.sync.dma_start(out=out_d[0:128, 0:NCH], in_=Acol)
        return

    # v-path bias row: bias''[o] = sum_c B[c] wvT[c,o] + bv[o], broadcast
    ps_br = PSS.tile([1, C], f32, tag="pscore", name="ps_br")
    for kc in range(NCH):
        nc.tensor.matmul(ps_br, lhsT=Bcol[:, kc:kc + 1], rhs=worig["v"][kc],
                         start=(kc == 0), stop=(kc == NCH - 1))
    brow = ST.tile([1, C], f32, tag="brow")
    nc.scalar.copy(out=brow, in_=ps_br)
    nc.vector.tensor_tensor(out=brow, in0=brow, in1=bvr, op=OP.add)
    ps_bb = PSS.tile([128, C], f32, tag="pscore", name="ps_bb")
    nc.tensor.matmul(ps_bb, lhsT=ones1, rhs=brow, start=True, stop=True)
    bvb = ST.tile([128, C], f32, tag="bvb")
    nc.vector.tensor_copy(out=bvb, in_=ps_bb)

    # q/k bias columns: bias'[o] = sum_c B[c] wT[c, o*] + b
    bq2 = ST.tile([128, NCH], f32, tag="bq2")
    bk2 = ST.tile([128, NCH], f32, tag="bk2")
    for nm, bcol, bout in (("q", bqc, bq2), ("k", bkc, bk2)):
        for oc in range(NCH):
            ps_b = PSS.tile([128, 1], f32, tag="pscore", name="ps_b")
            for kc in range(NCH):
                nc.tensor.matmul(ps_b,
                                 lhsT=worig[nm][kc][:, oc * 128:(oc + 1) * 128],
                                 rhs=Bcol[:, kc:kc + 1],
                                 start=(kc == 0), stop=(kc == NCH - 1))
            nc.vector.tensor_tensor(out=bout[:, oc:oc + 1], in0=ps_b,
                                    in1=bcol[:, oc:oc + 1], op=OP.add)

    _dump("dAcol", Acol)
    _dump("dBcol", Bcol)
    _dump("dbq2", bq2)
    _dump("dbvb", bvb)

    # fold A into wq/wk/wv (separate tiles; originals stay pristine)
    wfold = {}
    for nm in ("q", "k", "v"):
        wfold[nm] = []
        for kc in range(NCH):
            t = FP.tile([128, C], f32r, tag=f"f{nm}{kc}")
            nc.vector.tensor_scalar_mul(out=t, in0=worig[nm][kc],
                                        scalar1=Acol[:, kc:kc + 1])
            wfold[nm].append(t)

    if stage <= 1:
        nc.sync.dma_start(out=out_d[0:128, 0:NCH], in_=Acol)
        return

    # ================= main per-window pipeline =================
    def qk_group_emitters(w, q_sb, k_sb):
        base = w * N
        ems = []
        for oc in range(NCH):
            for dst, wf, bcol in ((q_sb, wfold["q"], bq2), (k_sb, wfold["k"], bk2)):
                for pc in range(2):
                    def em(dst=dst, wf=wf, bcol=bcol, oc=oc, pc=pc):
                        ps = PSP.tile([128, 512], f32, tag="pp", name="ps_qk")
                        for kc in range(NCH):
                            nc.tensor.matmul(
                                ps,
                                lhsT=wf[kc][:, oc * 128:(oc + 1) * 128],
                                rhs=x_sb[kc][:, base + pc * 512:base + (pc + 1) * 512],
                                start=(kc == 0), stop=(kc == NCH - 1))
                        nc.vector.tensor_scalar(
                            out=dst[oc][:, pc * 512:(pc + 1) * 512], in0=ps,
                            scalar1=bcol[:, oc:oc + 1], scalar2=None, op0=OP.add)
                    ems.append(em)
        return ems

    qk_tiles = []
    for w in range(NWIN):
        q_sb = [QK.tile([128, N], bf16, tag=f"q{kc}", name=f"q{kc}") for kc in range(NCH)]
        k_sb = [QK.tile([128, N], bf16, tag=f"k{kc}", name=f"k{kc}") for kc in range(NCH)]
        qk_tiles.append((q_sb, k_sb))

    pending = []      # deferred q/k projection emitters for the next window
    pending_wo = []   # deferred output-projection emitters from prior window
    for w in range(NWIN):
        base = w * N
        # --- projections (this window's remaining + allocate tiles) ---
        q_sb, k_sb = qk_tiles[w]
        for em in pending:
            em()
        pending = qk_group_emitters(w + 1, *qk_tiles[w + 1]) if w + 1 < NWIN else []
        if w == 0:
            for em in qk_group_emitters(0, q_sb, k_sb):
                em()
        # vt tiles: per-head 128-col blocks; even head h: [v_h | ones],
        # odd head h: [ones | v_h] -> one matmul per head gives out^T on one
        # partition half and softmax row-sums on the other.
        vt = []
        for mc in range(8):
            t = VT.tile([128, 1024], bf16, tag=f"vt{mc}")
            ps = PSP.tile([128, 512], f32, tag="pp")
            for kc in range(NCH):
                nc.tensor.matmul(
                    ps,
                    lhsT=x_sb[kc][:, base + mc * 128:base + (mc + 1) * 128],
                    rhs=wfold["v"][kc],
                    start=(kc == 0), stop=(kc == NCH - 1))
            ap8 = lambda a, off, step: bass.AP(
                tensor=a.tensor, offset=a.offset + off,
                ap=[a.ap[0], [step, 8], [1, 64]])
            # every head block is [v_h(64) | ones(64)]
            nc.vector.tensor_tensor(out=ap8(t, 0, 128), in0=ap8(ps, 0, 64),
                                    in1=ap8(bvb, 0, 64), op=OP.add)
            nc.vector.memset(ap8(t, 64, 128), 1.0)
            vt.append(t)

        if w == 0:
            _dump("dq0", q_sb[0], YP)
            _dump("dk0", k_sb[0], YP)
            _dump("dvt0", vt[0], YP)

        if stage <= 2:
            nc.sync.dma_start(out=out_d[0:128, w * N:w * N + 512],
                              in_=vt[0].bitcast(f32)[:, 0:512])
            continue

        # --- attention heads ---
        ao_sb = ([AO.tile([128, N], bf16, tag=f"ao{kc}", name=f"ao{kc}")
                  for kc in range(NCH)] if stage >= 4 else None)
        def scores_exp(h):
            ck, po = h // 2, (h % 2) * 64
            es_t = []
            for mc in range(8):
                ps_s = PSS.tile([128, N], f32, tag="pscore", name="ps_s")
                for nh in range(2):
                    nc.tensor.matmul(
                        ps_s[:, nh * 512:(nh + 1) * 512],
                        lhsT=k_sb[ck][po:po + 64, mc * 128:(mc + 1) * 128],
                        rhs=q_sb[ck][po:po + 64, nh * 512:(nh + 1) * 512],
                        start=True, stop=True)
                et = ES.tile([128, N], bf16, tag=f"es{mc}", name=f"es{mc}")
                nc.scalar.activation(out=et, in_=ps_s, func=FT.Exp,
                                     scale=SCALE)
                if dbg_d is not None and w == 0 and h == 0 and mc == 0:
                    _dump("des0", et, YP)
                es_t.append(et)
            return es_t

        def attn_v(h, es_t):
            ck, po = h // 2, (h % 2) * 64
            ps_av = PSA.tile([128, N], f32, tag="pav", name="ps_av")
            for mc in range(8):
                lhsT = vt[mc][:, h * 128:(h + 1) * 128]
                for nh in range(2):
                    nc.tensor.matmul(ps_av[:, nh * 512:(nh + 1) * 512],
                                     lhsT=lhsT,
                                     rhs=es_t[mc][:, nh * 512:(nh + 1) * 512],
                                     start=(mc == 0), stop=(mc == 7))
            # normalize: out = out_un * (1/rowsum)
            if dbg_d is not None and w == 0 and h == 0:
                for o in range(0, N, 512):
                    st = YP.tile([128, 512], f32, tag="y", name="dbgav")
                    nc.vector.tensor_copy(out=st, in_=ps_av[:, o:o + 512])
                    nc.sync.dma_start(out=dbg_d["dav"][:, o:o + 512], in_=st)
            rr_t = RR.tile([64, N], f32, tag="rraw", name="rr_t")
            ri_t = RR.tile([64, N], f32, tag="rinv", name="ri_t")
            # rowsums sit at psum partitions 64:128 -> shift down (1-input
            # copies may cross partitions; reciprocal_approx_fast may not).
            nc.vector.tensor_copy(out=rr_t, in_=ps_av[64:128, :])
            nc.vector.reciprocal_approx_fast(out=ri_t, in_=rr_t)
            # 2-input mult with matched input partitions; output partition
            # offset po is legal (verified on HW).
            nc.vector.tensor_tensor(out=ao_sb[ck][po:po + 64, :],
                                    in0=ps_av[0:64, :], in1=ri_t,
                                    op=OP.mult)
            if dbg_d is not None and w == 0 and h == 0:
                _dump("drr", rr_t)
                _dump("dri", ri_t)

        # software pipeline: emit scores/exp(h) before attn_v(h-1) so the PE
        # fills the exp-wait gap of head h-1 with head h's score matmuls.
        prev = None
        for h in range(NHEADS):
            es_t = scores_exp(h)
            if stage > 3 and h >= 2:
                for _ in range(3):
                    if pending:
                        pending.pop(0)()
                if pending_wo:
                    pending_wo.pop(0)()
                if h == NHEADS - 1:
                    while pending_wo:
                        pending_wo.pop(0)()
            if stage <= 3:
                if h == NHEADS - 1:
                    st3 = YP.tile([128, 512], f32, tag="y", name="st3dump")
                    nc.vector.tensor_copy(out=st3, in_=es_t[0][:, 0:512])
                    nc.sync.dma_start(out=out_d[0:128, w * N:w * N + 512],
                                      in_=st3)
                continue
            if prev is not None:
                attn_v(*prev)
            prev = (h, es_t)
        if prev is not None:
            attn_v(*prev)

        if w == 0 and ao_sb is not None:
            _dump("dao0", ao_sb[0])

        if stage <= 4:
            if stage == 4:
                st4 = YP.tile([128, 512], f32, tag="y", name="st4dump")
                nc.vector.tensor_copy(out=st4, in_=ao_sb[0][:, 0:512])
                nc.sync.dma_start(out=out_d[0:128, w * N:w * N + 512], in_=st4)
            continue

        # --- output projection + residual (deferred into next window) ---
        def wo_emitters(base, ao_sb):
            ems = []
            for oc in range(NCH):
                for nh in range(2):
                    def em(oc=oc, nh=nh, base=base, ao_sb=ao_sb):
                        ps_y = PSP.tile([128, 512], f32, tag="pp", name="ps_y")
                        for kc in range(NCH):
                            nc.tensor.matmul(
                                ps_y,
                                lhsT=worig["o"][kc][:, oc * 128:(oc + 1) * 128],
                                rhs=ao_sb[kc][:, nh * 512:(nh + 1) * 512],
                                start=(kc == 0), stop=(kc == NCH - 1))
                        o_t = OS.tile([128, 512], f32, tag="osb", name="o_t")
                        nc.vector.scalar_tensor_tensor(
                            out=o_t, in0=ps_y, scalar=boc[:, oc:oc + 1],
                            in1=x_sb[oc].bitcast(f32)[:, base + nh * 512:base + (nh + 1) * 512],
                            op0=OP.add, op1=OP.add)
                        nc.sync.dma_start(
                            out=out_d[oc * 128:(oc + 1) * 128,
                                      base + nh * 512:base + (nh + 1) * 512],
                            in_=o_t)
                    ems.append(em)
            return ems

        pending_wo.extend(wo_emitters(base, ao_sb))
        if w == NWIN - 1:
            for em in pending_wo:
                em()
            pending_wo = []


# ---------------- host-side marshalling ----------------

def _rasterize(xb_half):
    """[C, 32, 64] -> [C, 2048] in (window, row, col) raster order."""
    return np.ascontiguousarray(
        xb_half.reshape(C, WS, 2, WS).transpose(0, 2, 1, 3).reshape(C, NPOS))


def _unrasterize(y):
    """[C, 2048] -> [C, 32, 64]."""
    return y.reshape(C, 2, WS, WS).transpose(0, 2, 1, 3).reshape(C, WS, W)


_NC_CACHE = {}


def _get_nc(reps=1):
    if reps not in _NC_CACHE:
        _NC_CACHE[reps] = build_kernel(reps)
    return _NC_CACHE[reps]


def make_in_maps(x, norm_scale, norm_bias, wq, bq, wk, bk, wv, bv, wo, bo):
    x = np.asarray(x, dtype=np.float32)
    cols = lambda v: np.ascontiguousarray(
        np.asarray(v, np.float32).reshape(NCH, 128).T)
    G = np.zeros((128, 8), np.float32)
    for p in range(128):
        G[p, p // 16] = 1.0
    shared = {
        "wqT": np.ascontiguousarray(np.asarray(wq, np.float32).T),
        "wkT": np.ascontiguousarray(np.asarray(wk, np.float32).T),
        "wvT": np.ascontiguousarray(np.asarray(wv, np.float32).T),
        "woT": np.ascontiguousarray(np.asarray(wo, np.float32).T),
        "gscale": cols(norm_scale), "gbias": cols(norm_bias),
        "bq": cols(bq), "bk": cols(bk), "bo": cols(bo),
        "bv": np.asarray(bv, np.float32).reshape(1, C).copy(),
        "G": G, "Gt": np.ascontiguousarray(G.T),
    }
    in_maps = []
    for c in range(NCORES):
        b, hi = c // 2, c % 2
        xm = _rasterize(x[b, :, hi * WS:(hi + 1) * WS, :])
        xo = _rasterize(x[b, :, (1 - hi) * WS:(1 - hi + 1) * WS, :])
        in_maps.append({"xm": xm, "xo": xo, **shared})
    return in_maps


def kernel(**inputs):
    nc = _get_nc(1)
    in_maps = make_in_maps(**inputs)
    res = run_bass_kernel_spmd(nc, in_maps, list(range(NCORES)))
    out = np.empty((B, C, H, W), np.float32)
    for c in range(NCORES):
        b, hi = c // 2, c % 2
        out[b, :, hi * WS:(hi + 1) * WS, :] = _unrasterize(res.results[c]["out"])
    return out
